# revision 40
# baseline (speedup 1.0000x reference)
"""Distributed CAP-memory loss kernel for 8 TRN2 NeuronCores.

Problem (see reference): given unit-norm features [B=256, D=2048] and a
memory bank [6, 2000, 2048], compute
  loss = sum_cam mean_cam(per-camera proxy CE)
       + 0.5 * sum_cam mean_cam(assoc loss over 6 positives + 50 hard negatives)

Distribution strategy (column/class sharding, interleaved):
  The 12000 memory rows are split so core k owns columns
  {j*2000 + k*250 + r : j in [0,6), r in [0,250)} -- an identical 250-wide
  slice of every camera block, so all 8 cores run the same SPMD program on
  1500 columns each:
    * sims_local = feats @ memT_local              (PE, bf16 -> f32 PSUM)
    * per-camera-block partial sum(exp(20*sims))   (ACT, accum)
    * top-16 of each 500-col chunk                 (DVE max8 + match_replace)
      -> 48 candidates per core, 384 per row globally
  The host merges the per-core stats ([256, 54] each): removes the (host
  computed) positives from the candidate lists, takes the global top-50
  with an exactness certificate (see host_combine) and an exact fallback,
  log-sum-exp combines, segment sums -> scalar loss.

memT/featsT are host-packed so each SBUF partition's data is one
contiguous DRAM run, streamed in ~0.5MB pieces round-robin over the three
DMA-capable queues in PE-consumption order.
"""

import os
import sys
import types

import numpy as np

# ---------------------------------------------------------------- constants
B = 256          # batch
D = 2048         # feature dim
NCAMS = 6
C = 2000         # classes per camera
NG = NCAMS * C   # 12000 global columns
M = 8            # cores
W = C // M       # 250: per-core slice width inside each camera block
NL = NCAMS * W   # 1500 local columns per core
P = 128          # partitions
KO = D // P      # 16 contraction chunks
BT = B // P      # 2 batch tiles
NT = 3           # epilogue column chunks
NCHUNK = NL // NT  # 500
KR = 4           # ko-range DMA pieces per nt phase (4 ko each)
KPR = KO // KR
FK = 4           # featsT DMA pieces (4 ko each)
BETA = 0.05
INV_BETA = 1.0 / BETA  # 20.0
BG_KNN = 50
KITC = 2         # top-8 iterations per 500-col chunk -> 16 cand/chunk
NCAND = KITC * 8 * NT  # 48 candidates per core
REPL_VAL = -30000.0
POS_TOL = 1e-3   # host-side positive-candidate matching tolerance
OUTC = NCAND + NCAMS  # 48 topk | 6 sumexp

LAST_EXEC_NS = None
FALLBACK_COUNT = 0
_NC_CACHE = {}


def _install_axon_ntff_hook():
    """The agent image's antenv lacks axon_hooks; synthesize it so
    run_bass_kernel_spmd(trace=True) can capture NTFF profiles."""
    if "antenv.axon_hooks" in sys.modules:
        return
    mod = types.ModuleType("antenv.axon_hooks")
    state = {"hook": None}
    mod.set_axon_ntff_profile_hook = lambda h: state.__setitem__("hook", h)
    mod.get_axon_ntff_profile_hook = lambda: state["hook"]
    sys.modules["antenv.axon_hooks"] = mod
    try:
        import antenv

        antenv.axon_hooks = mod
    except Exception:
        pass
    try:
        from trn_agent_boot.trn_boot import _ntff_profile_via_ctypes

        hook = _ntff_profile_via_ctypes("/opt/axon/libaxon_pjrt.so")
        if hook is not None:
            mod.set_axon_ntff_profile_hook(hook)
    except Exception:
        pass


def build_nc(mm_dtype_name: str = "bfloat16"):
    """Build + compile the single SPMD Bass program shared by all 8 cores."""
    import concourse.bacc as bacc
    import concourse.mybir as mybir
    import concourse.tile as tile

    f32 = mybir.dt.float32
    mm_dt = getattr(mybir.dt, mm_dtype_name)
    A = mybir.AluOpType
    AF = mybir.ActivationFunctionType

    nc = bacc.Bacc(
        "TRN2",
        target_bir_lowering=False,
        debug=False,
        enable_asserts=False,
        num_devices=M,
    )

    featsT_d = nc.dram_tensor("featsT", [P, KO * B], mm_dt, kind="ExternalInput")
    memT_d = nc.dram_tensor("memT", [P, KO * NL], mm_dt, kind="ExternalInput")
    out_d = nc.dram_tensor("out", [B, OUTC], f32, kind="ExternalOutput")

    with tile.TileContext(nc) as tc:
        with (
            tc.tile_pool(name="big", bufs=1) as big,
            tc.tile_pool(name="work", bufs=BT) as work,
            tc.tile_pool(name="scr", bufs=4) as scr,
            tc.tile_pool(name="psum", bufs=BT * NT, space="PSUM") as psum,
        ):
            featsT_sb = big.tile([P, KO * B], mm_dt)
            memT_sb = big.tile([P, KO * NL], mm_dt)
            queues = [nc.sync, nc.scalar, nc.gpsimd]

            def feats_piece(qi, klo, khi):
                fsl = slice(klo * B, khi * B)
                queues[qi].dma_start(featsT_sb[:, fsl], featsT_d[:, fsl])

            def mem_piece(qi, nt, klo, khi):
                # [ko-range x 500col] block: strided over ko, contiguous 500
                src = memT_d[:].rearrange("p (ko n) -> p ko n", n=NL)[
                    :, klo:khi, nt * NCHUNK : (nt + 1) * NCHUNK
                ]
                dst = memT_sb[:].rearrange("p (ko n) -> p ko n", n=NL)[
                    :, klo:khi, nt * NCHUNK : (nt + 1) * NCHUNK
                ]
                queues[qi].dma_start(dst, src)

            # Streaming order: all of nt0, then featsT, then nt1/nt2.
            # PSUM-feeding DMA runs ~50% slower while the PE streams (SBUF
            # port contention), and the PE's busy time is fixed, so the
            # fastest schedule pre-buffers enough that the PE -- gated here
            # by featsT arriving after nt0 -- never stalls once started.
            mem_piece(0, 0, 0, 6)    # sync:   nt0 ko0-5
            mem_piece(1, 0, 6, 11)   # scalar: nt0 ko6-10
            mem_piece(2, 0, 11, 16)  # gpsimd: nt0 ko11-15
            feats_piece(0, 0, 6)     # sync:   feats ko0-5
            feats_piece(1, 6, 11)    # scalar: feats ko6-10
            feats_piece(2, 11, 16)   # gpsimd: feats ko11-15
            mem_piece(0, 1, 0, 6)    # sync:   nt1 ko0-5
            mem_piece(1, 1, 6, 11)   # scalar: nt1 ko6-10
            mem_piece(2, 1, 11, 16)  # gpsimd: nt1 ko11-15
            mem_piece(0, 2, 0, 6)    # sync:   nt2 ko0-5
            mem_piece(1, 2, 6, 11)   # scalar: nt2 ko6-10
            mem_piece(2, 2, 11, 16)  # gpsimd: nt2 ko11-15

            simsb = [
                work.tile([P, NL], f32, tag="simsb", name=f"simsb{b}")
                for b in range(BT)
            ]
            outs = [
                work.tile([P, OUTC], f32, tag="outs", name=f"outs{b}")
                for b in range(BT)
            ]
            pstiles = [
                psum.tile([P, NCHUNK], f32, tag="ps", name=f"ps{b}_{n}")
                for b in range(BT)
                for n in range(NT)
            ]

            def mm(bt, nt, ko):
                nc.tensor.matmul(
                    pstiles[bt * NT + nt][:],
                    featsT_sb[:, ko * B + bt * P : ko * B + (bt + 1) * P],
                    memT_sb[
                        :, ko * NL + nt * NCHUNK : ko * NL + (nt + 1) * NCHUNK
                    ],
                    start=(ko == 0),
                    stop=(ko == KO - 1),
                )

            def epilogue(bt, nt):
                ps = pstiles[bt * NT + nt]
                nsl = slice(nt * NCHUNK, (nt + 1) * NCHUNK)
                # PSUM -> SBUF (ACT) so the DVE top-k can run on it
                nc.scalar.copy(simsb[bt][:, nsl], ps[:])
                # top-16 of this 500-col chunk (positives removed on host)
                for it in range(KITC):
                    col = (nt * KITC + it) * 8
                    nc.vector.max(
                        out=outs[bt][:, col : col + 8],
                        in_=simsb[bt][:, nsl],
                    )
                    if it < KITC - 1:
                        nc.vector.match_replace(
                            out=simsb[bt][:, nsl],
                            in_to_replace=outs[bt][:, col : col + 8],
                            in_values=simsb[bt][:, nsl],
                            imm_value=REPL_VAL,
                        )
                for s in range(NCHUNK // W):
                    j = nt * (NCHUNK // W) + s
                    pse = ps[:, s * W : (s + 1) * W]
                    # per-camera-block sum(exp(sims/beta)); sims in (-1,1)
                    # so exp(20*sims) stays in f32 range without bias
                    et = scr.tile([P, W], f32, tag="exp")
                    nc.scalar.activation(
                        et[:],
                        pse,
                        AF.Exp,
                        scale=INV_BETA,
                        accum_out=outs[bt][:, NCAND + j : NCAND + j + 1],
                    )

            for nt in range(NT):
                for kr in range(KR):
                    for bt in range(BT):
                        for ko in range(kr * KPR, (kr + 1) * KPR):
                            mm(bt, nt, ko)
                for bt in range(BT):
                    epilogue(bt, nt)

            for bt in range(BT):
                queues[1 + bt].dma_start(
                    out_d[bt * P : (bt + 1) * P, :], outs[bt][:]
                )

    nc.compile()
    return nc


def get_nc(mm_dtype_name: str = None):
    if mm_dtype_name is None:
        mm_dtype_name = os.environ.get("CAP_MM_DTYPE", "bfloat16")
    if mm_dtype_name not in _NC_CACHE:
        _NC_CACHE[mm_dtype_name] = build_nc(mm_dtype_name)
    return _NC_CACHE[mm_dtype_name]


def _mm_np_dtype():
    name = os.environ.get("CAP_MM_DTYPE", "bfloat16")
    if name == "bfloat16":
        import ml_dtypes

        return np.dtype(ml_dtypes.bfloat16)
    return np.dtype(np.float32)


def shard_cols(k: int) -> np.ndarray:
    """Global memory-bank columns owned by core k."""
    return (
        np.arange(NCAMS)[:, None] * C + k * W + np.arange(W)[None, :]
    ).reshape(-1)


def pack_featsT(features: np.ndarray) -> np.ndarray:
    """[B, D] -> [P, KO*B] with row p holding feats.T[ko*128+p, :] runs."""
    arr = features.T.reshape(KO, P, B).transpose(1, 0, 2).reshape(P, KO * B)
    return np.ascontiguousarray(arr).astype(_mm_np_dtype())


def pack_memT(mem_flat: np.ndarray, cols: np.ndarray) -> np.ndarray:
    """[NG, D] -> [P, KO*NL] packed like pack_featsT for this core's cols."""
    arr = (
        mem_flat[cols].T.reshape(KO, P, NL).transpose(1, 0, 2).reshape(P, KO * NL)
    )
    return np.ascontiguousarray(arr).astype(_mm_np_dtype())


def _loss_from_parts(pos_logits, lse_block, top50, cams):
    rows = np.arange(B)
    ce = lse_block[rows, cams] - pos_logits[rows, cams]
    logits = np.concatenate([pos_logits, INV_BETA * top50], axis=1)
    mx = logits.max(axis=1, keepdims=True)
    lse56 = mx[:, 0] + np.log(np.exp(logits - mx).sum(axis=1))
    assoc = lse56 - pos_logits.sum(axis=1) / NCAMS

    counts = np.bincount(cams, minlength=NCAMS).astype(np.float64)
    ce_sum = np.bincount(cams, weights=ce, minlength=NCAMS)
    as_sum = np.bincount(cams, weights=assoc, minlength=NCAMS)
    safe = np.maximum(counts, 1.0)
    present = counts > 0
    return np.sum(np.where(present, ce_sum / safe, 0.0)) + np.sum(
        np.where(present, 0.5 * as_sum / safe, 0.0)
    )


def host_combine(outs, features, memory, cams, labels):
    """outs: [M, B, OUTC] device results."""
    global FALLBACK_COUNT
    cand = outs[:, :, :NCAND].astype(np.float64)  # [M, B, 48]
    sexp = outs[:, :, NCAND:].astype(np.float64)  # [M, B, 6]

    s_block = sexp.sum(axis=0)  # [B, 6] sum(exp(20*sims)) per camera block
    lse_block = np.log(s_block)  # logsumexp of own-camera logits

    # positives: one dot product per (row, camera) -- 6.3 MFLOP on host
    feats64 = np.asarray(features, np.float64)
    pos_vals = np.einsum(
        "bd,jbd->bj",
        feats64,
        np.asarray(memory, np.float64)[:, labels, :],
        optimize=True,
    )  # [B, 6]

    # [B, M*NT, 16] per-(core,chunk) candidate lists
    percl = (
        cand.transpose(1, 0, 2)
        .reshape(B, M, NT, KITC * 8)
        .reshape(B, M * NT, KITC * 8)
        .copy()
    )
    cmin_raw = percl.min(axis=2)  # pre-drop floor per (core,chunk)

    # Remove positives from the candidate lists.  Positive (i, j) can only
    # appear on core labels[i]//W, chunk j//2; drop the closest value
    # within POS_TOL (missing a true positive would corrupt the hard
    # negatives; over-dropping a near-equal genuine value is harmless).
    own_core = labels // W  # [B]
    for j in range(NCAMS):
        cl = own_core * NT + j // 2  # [B] chunk-list index
        lists = percl[np.arange(B), cl]  # [B, 16] (view via fancy idx: copy)
        diff = np.abs(lists - pos_vals[:, j : j + 1])
        am = diff.argmin(axis=1)
        hit = diff[np.arange(B), am] < POS_TOL
        lists[hit, am[hit]] = -np.inf
        percl[np.arange(B), cl] = lists

    flat = percl.reshape(B, -1)
    top50 = -np.partition(-flat, BG_KNN - 1, axis=1)[:, :BG_KNN]
    t50 = top50[:, BG_KNN - 1]  # [B] 50th largest of the union

    # Exactness certificate: every (core,chunk)'s smallest extracted
    # candidate must lie strictly below the union's 50th value, proving no
    # unseen value could reach the global top-50.
    bad = (cmin_raw >= t50[:, None]).any(axis=1)
    if bad.any():
        # Exact fallback for insufficient rows: recompute on the host.
        FALLBACK_COUNT += int(bad.sum())
        mem_flat = np.asarray(memory, np.float32).reshape(NG, D)
        idx = np.nonzero(bad)[0]
        sims = np.asarray(features, np.float32)[idx] @ mem_flat.T
        colsg = np.arange(NG)
        for p, i in enumerate(idx):
            row = sims[p].astype(np.float64)
            row[colsg % C == labels[i]] = -np.inf
            top50[i] = -np.sort(-row)[:BG_KNN]

    return np.float32(
        _loss_from_parts(INV_BETA * pos_vals, lse_block, top50, cams)
    )


def kernel(features, memory, cams, labels, trace: bool = None):
    global LAST_EXEC_NS
    _install_axon_ntff_hook()
    from concourse.bass_utils import run_bass_kernel_spmd

    features = np.asarray(features, dtype=np.float32)
    memory = np.asarray(memory, dtype=np.float32)
    cams = np.asarray(cams).astype(np.int64)
    labels = np.asarray(labels).astype(np.int64)

    nc = get_nc()

    mem_flat = memory.reshape(NG, D)
    featsT = pack_featsT(features)
    in_maps = [
        {"featsT": featsT, "memT": pack_memT(mem_flat, shard_cols(k))}
        for k in range(M)
    ]

    if trace is None:
        trace = os.environ.get("CAP_TRACE", "1") == "1"
    res = run_bass_kernel_spmd(
        nc, in_maps, core_ids=list(range(M)), trace=trace
    )
    if res.exec_time_ns is not None:
        LAST_EXEC_NS = res.exec_time_ns

    outs = np.stack([r["out"] for r in res.results])  # [M, B, OUTC]
    return np.asarray(
        host_combine(outs, features, memory, cams, labels), dtype=np.float32
    )


# ------------------------------------------------------------------ helpers
def expected_core_out(features, memory, labels, k: int) -> np.ndarray:
    """Numpy model of what core k's device program should output [B, OUTC]."""
    mem_flat = np.asarray(memory, np.float32).reshape(NG, D)
    cols = shard_cols(k)
    sims = np.asarray(features, np.float32) @ mem_flat[cols].T  # [B, NL]
    out = np.zeros((B, OUTC), np.float32)
    for j in range(NCAMS):
        jsl = slice(j * W, (j + 1) * W)
        out[:, NCAND + j] = np.exp(
            INV_BETA * sims[:, jsl].astype(np.float64)
        ).sum(axis=1)
    for nt in range(NT):
        chunk = sims[:, nt * NCHUNK : (nt + 1) * NCHUNK]
        srt = -np.sort(-chunk, axis=1)
        out[:, nt * KITC * 8 : (nt + 1) * KITC * 8] = srt[:, : KITC * 8]
    return out
